# revision 1
# baseline (speedup 1.0000x reference)
"""Trainium2 Bass kernel for AttnBlock (rmsnorm -> qkv -> block-causal frame
attention -> output proj -> residual).

Sharding: 8 cores; core i owns the i-th 128-token slice of every frame
(8 query chunks of 128 tokens).  Query chunk s (from frame s) attends exactly
frames 0..s, so the attention spans are compile-time constants, identical on
every core -> clean SPMD with no masks and no dynamic addressing.  All
per-core variation is carried by the x_own input slice.

Every core computes K/V for the full 8192-token sequence (replicating that
compute is far cheaper than intra-chip collectives).  Matmul inputs are bf16
(fp32 matmul is 4x slower on PE); accumulation is fp32 in PSUM.  Softmax skips
the max-subtraction (scores are O(1) for this problem's scale) and the row
sums are computed with a ones-vector matmul on the tensor engine; the 1/sum
normalization is applied after the attn@V contraction.
"""

import math
import os
import sys

import numpy as np

for _p in ("/opt/trn_rl_repo",):
    if _p not in sys.path:
        sys.path.insert(0, _p)

import ml_dtypes  # noqa: E402

import concourse.bass as bass  # noqa: E402
import concourse.tile as tile  # noqa: E402
from concourse import bacc  # noqa: E402
from concourse import mybir  # noqa: E402
from concourse.bass_utils import run_bass_kernel_spmd  # noqa: E402

BF16 = mybir.dt.bfloat16
F32 = mybir.dt.float32

B, C, NF, H, W = 1, 512, 8, 32, 32
NHW = H * W          # 1024 tokens per frame
SEQ = NF * NHW       # 8192
NCORES = 8
P = 128              # partitions
CC = C // P          # 4 channel chunks
NT = NHW // P        # 8 key tiles per frame
NQ = NHW // P        # 8 query chunks of 128 per core (one per frame)
SQRT_C = math.sqrt(C)
INV_SQRT_C = 1.0 / SQRT_C

LAST_RESULTS = None  # BassKernelResults of the most recent run (for test.py)


def _ensure_axon_hooks():
    """bass_utils' trace path imports antenv.axon_hooks, which is absent from
    some container snapshots.  Provide the tiny registry (and wire the ctypes
    NTFF hook from trn_agent_boot when available) so tracing degrades
    gracefully instead of crashing."""
    import types

    try:
        import antenv.axon_hooks  # noqa: F401

        return
    except Exception:
        pass
    try:
        import antenv
    except Exception:
        antenv = types.ModuleType("antenv")
        sys.modules["antenv"] = antenv
    mod = types.ModuleType("antenv.axon_hooks")
    _h = [None]
    mod.set_axon_ntff_profile_hook = lambda hook: _h.__setitem__(0, hook)
    mod.get_axon_ntff_profile_hook = lambda: _h[0]
    sys.modules["antenv.axon_hooks"] = mod
    antenv.axon_hooks = mod
    try:
        from trn_agent_boot.trn_boot import _ntff_profile_via_ctypes

        hook = _ntff_profile_via_ctypes("/opt/axon/libaxon_pjrt.so")
        if hook is not None:
            mod.set_axon_ntff_profile_hook(hook)
    except Exception:
        pass


def _bcast(ap, p=P):
    """AP that reads a DRAM row and replicates it across p partitions."""
    return bass.AP(tensor=ap.tensor, offset=ap.offset, ap=[[0, p], *list(ap.ap)])


def _build_nc():
    nc = bacc.Bacc()

    xf = nc.declare_dram_parameter("xf", [C, SEQ], BF16, isOutput=False)
    xo = nc.declare_dram_parameter("xo", [C, NHW], F32, isOutput=False)
    wqT = nc.declare_dram_parameter("wqT", [C, C], BF16, isOutput=False)
    wkT = nc.declare_dram_parameter("wkT", [C, C], BF16, isOutput=False)
    wvT = nc.declare_dram_parameter("wvT", [C, C], BF16, isOutput=False)
    woT = nc.declare_dram_parameter("woT", [C, C], BF16, isOutput=False)
    gamma = nc.declare_dram_parameter("gamma", [C], F32, isOutput=False)
    bq = nc.declare_dram_parameter("bq", [C], F32, isOutput=False)
    bk = nc.declare_dram_parameter("bk", [C], F32, isOutput=False)
    bv = nc.declare_dram_parameter("bv", [C], F32, isOutput=False)
    bo = nc.declare_dram_parameter("bo", [C], F32, isOutput=False)
    out = nc.declare_dram_parameter("out", [C, NHW], F32, isOutput=True)

    with tile.TileContext(nc) as tc:
        _emit(tc, xf, xo, wqT, wkT, wvT, woT, gamma, bq, bk, bv, bo, out)
    return nc


def _emit(tc, xf, xo, wqT, wkT, wvT, woT, gamma, bq, bk, bv, bo, out):
    nc = tc.nc
    Act = mybir.ActivationFunctionType
    Alu = mybir.AluOpType

    with (
        tc.tile_pool(name="dram", bufs=1, space="DRAM") as drp,
        tc.tile_pool(name="singles", bufs=1) as singles,
    ):
        # ---- DRAM scratch ----
        vt_dram = drp.tile([NF * NT, P, C], BF16)   # v in [token, channel] tiles
        r_dram = drp.tile([NF + 1, 1, NHW], F32)       # rms scale rows (frames + own)
        rc_dram = drp.tile([1, NHW], F32)              # softmax 1/sum row (own queries)

        # ---- constants & weights in SBUF ----
        _wdefer = []

        def _wsb(wT):
            t = singles.tile([P, CC, C], BF16, tag=f"w_{wT.name}")
            _wdefer.append((t, wT))
            return t

        wq_sb, wk_sb, wv_sb, wo_sb = _wsb(wqT), _wsb(wkT), _wsb(wvT), _wsb(woT)

        def _col(v):
            t = singles.tile([P, CC], F32, tag=f"col_{v.name}")
            nc.sync.dma_start(t[:], v[:].rearrange("(cc p) -> p cc", p=P))
            return t

        gammaCol, bqCol, bkCol, boCol = _col(gamma), _col(bq), _col(bk), _col(bo)
        gsCol = singles.tile([P, CC], F32)          # gamma * sqrt(C)
        nc.vector.tensor_scalar_mul(gsCol[:], gammaCol[:], float(SQRT_C))

        bvB = singles.tile([P, C], F32)             # bv broadcast along partitions
        nc.sync.dma_start(bvB[:], _bcast(bv[:]))

        ones_bf = singles.tile([P, 1], BF16)
        nc.vector.memset(ones_bf[:], 1.0)
        eps_row = singles.tile([1, 1], F32)
        nc.vector.memset(eps_row[:], 1e-24)

        # ---- big persistent tensors ----
        K_sb = singles.tile([P, CC, SEQ], BF16)     # keys  [c_chunk, seq]
        Q_sb = singles.tile([P, CC, NHW], BF16)     # own queries
        xo_sb = singles.tile([P, CC, NHW], F32)     # own x (residual)

        # ================= phase A: norm + projections =================
        with (
            tc.tile_pool(name="stage", bufs=2) as stage,
            tc.tile_pool(name="hpool", bufs=3) as hpool,
            tc.tile_pool(name="rbpool", bufs=2) as rbpool,
            tc.tile_pool(name="rows", bufs=3) as rows,
            tc.tile_pool(name="vstage", bufs=4) as vstage,
            tc.tile_pool(name="ppA", bufs=5, space="PSUM") as ppA,
            tc.tile_pool(name="ppS", bufs=3, space="PSUM") as ppS,
        ):
            # ---- A0: norm reciprocal rows for all frames (+ own), half-frame
            # granularity with dedicated staging so it runs far ahead ----
            for step in range(NF + 1):
                own = step == 1
                f = NF if own else (0 if step == 0 else step - 1)
                if own:
                    nc.sync.dma_start(
                        xo_sb[:], xo[:].rearrange("(cc p) s -> p cc s", p=P)
                    )
                for pc in range(2):
                    if own:
                        x0_in = xo_sb[:, :, pc * 512 : (pc + 1) * 512]
                    else:
                        xt0 = stage.tile([P, CC, 512], BF16, tag="xt0")
                        nc.sync.dma_start(
                            xt0[:],
                            xf[:].rearrange("(cc p) s -> p cc s", p=P)[
                                :, :, f * NHW + pc * 512 : f * NHW + (pc + 1) * 512
                            ],
                        )
                        x0_in = xt0
                    x2 = stage.tile([P, CC, 512], BF16, tag="x2")
                    nc.scalar.activation(x2[:], x0_in[:], Act.Square)
                    ps = ppS.tile([1, 512], F32)
                    for cc in range(CC):
                        nc.tensor.matmul(
                            ps[:],
                            lhsT=ones_bf[:],
                            rhs=x2[:, cc, :],
                            start=(cc == 0),
                            stop=(cc == CC - 1),
                        )
                    nrm = rows.tile([1, 512], F32, tag="nrm")
                    nc.scalar.activation(nrm[:], ps[:], Act.Sqrt, bias=eps_row[:])
                    rrow = rows.tile([1, 512], F32, tag="rrow")
                    nc.vector.reciprocal_approx_fast(out=rrow[:], in_=nrm[:])
                    nc.sync.dma_start(
                        r_dram[f, :, pc * 512 : (pc + 1) * 512], rrow[:]
                    )

            for t, wT in _wdefer:
                nc.sync.dma_start(t[:], wT[:].rearrange("(cc p) o -> p cc o", p=P))

            # ---- A1: h + projections per frame ----
            for step in range(NF + 1):
                own = step == 0
                f = NF if own else step - 1

                if own:
                    xt_in = xo_sb  # fp32, already loaded
                else:
                    xt = stage.tile([P, CC, NHW], BF16, tag="xt")
                    nc.sync.dma_start(
                        xt[:],
                        xf[:].rearrange("(cc p) s -> p cc s", p=P)[
                            :, :, f * NHW : (f + 1) * NHW
                        ],
                    )
                    xt_in = xt

                rB = rbpool.tile([P, NHW], F32)
                nc.sync.dma_start(rB[:], _bcast(r_dram[f, 0]))

                # h = bf16(x * gamma_c * sqrtC / nrm_s)
                if own:
                    xbf = stage.tile([P, CC, NHW], BF16, tag="xt")
                    nc.vector.tensor_copy(out=xbf[:], in_=xo_sb[:])
                    xsrc = xbf
                else:
                    xsrc = xt_in
                ht = hpool.tile([P, CC, NHW], BF16)
                for cc in range(CC):
                    nc.vector.scalar_tensor_tensor(
                        out=ht[:, cc, :],
                        in0=xsrc[:, cc, :],
                        scalar=gsCol[:, cc : cc + 1],
                        in1=rB[:],
                        op0=Alu.mult,
                        op1=Alu.mult,
                    )

                if os.environ.get("BASS_DEBUG") == "h" and (not own) and f == 0:
                    hd = hpool.tile([P, CC, NHW], F32, name="hd")
                    nc.vector.tensor_copy(out=hd[:], in_=ht[:])
                    nc.sync.dma_start(out[:].rearrange("(cc p) s -> p cc s", p=P), hd[:])

                if own:
                    # Q projection
                    for oc in range(CC):
                        for pc in range(2):
                            psq = ppA.tile([P, 512], F32, tag="ps", name="psq")
                            for cc in range(CC):
                                nc.tensor.matmul(
                                    psq[:],
                                    lhsT=wq_sb[:, cc, oc * P : (oc + 1) * P],
                                    rhs=ht[:, cc, pc * 512 : (pc + 1) * 512],
                                    start=(cc == 0),
                                    stop=(cc == CC - 1),
                                )
                            nc.scalar.activation(
                                Q_sb[:, oc, pc * 512 : (pc + 1) * 512],
                                psq[:],
                                Act.Identity,
                                bias=bqCol[:, oc : oc + 1],
                            )
                else:
                    # K projection for this frame
                    for oc in range(CC):
                        for pc in range(2):
                            psk = ppA.tile([P, 512], F32, tag="ps", name="psk")
                            for cc in range(CC):
                                nc.tensor.matmul(
                                    psk[:],
                                    lhsT=wk_sb[:, cc, oc * P : (oc + 1) * P],
                                    rhs=ht[:, cc, pc * 512 : (pc + 1) * 512],
                                    start=(cc == 0),
                                    stop=(cc == CC - 1),
                                )
                            nc.scalar.activation(
                                K_sb[:, oc, f * NHW + pc * 512 : f * NHW + (pc + 1) * 512],
                                psk[:],
                                Act.Identity,
                                bias=bkCol[:, oc : oc + 1],
                            )
                    # V projection, [token, channel] layout, spilled to DRAM
                    for t in range(NT):
                        psv = ppA.tile([P, 512], F32, tag="ps", name="psv")
                        for cc in range(CC):
                            nc.tensor.matmul(
                                psv[:],
                                lhsT=ht[:, cc, t * P : (t + 1) * P],
                                rhs=wv_sb[:, cc, :],
                                start=(cc == 0),
                                stop=(cc == CC - 1),
                            )
                        vt_s = vstage.tile([P, C], BF16)
                        nc.vector.scalar_tensor_tensor(
                            out=vt_s[:],
                            in0=psv[:],
                            scalar=1.0,
                            in1=bvB[:],
                            op0=Alu.mult,
                            op1=Alu.add,
                        )
                        nc.sync.dma_start(vt_dram[f * NT + t], vt_s[:])

        dbg = os.environ.get("BASS_DEBUG", "")
        if dbg in ("K", "Q", "vt", "r", "h"):
            with tc.tile_pool(name="dbgp", bufs=1) as dbgp:
                if dbg == "K":
                    d = dbgp.tile([P, CC, NHW], F32, name="d")
                    nc.vector.tensor_copy(out=d[:], in_=K_sb[:, :, :NHW])
                    nc.sync.dma_start(out[:].rearrange("(cc p) s -> p cc s", p=P), d[:])
                elif dbg == "Q":
                    d = dbgp.tile([P, CC, NHW], F32, name="d")
                    nc.vector.tensor_copy(out=d[:], in_=Q_sb[:])
                    nc.sync.dma_start(out[:].rearrange("(cc p) s -> p cc s", p=P), d[:])
                elif dbg == "vt":
                    for t in range(NT):
                        vtt = dbgp.tile([P, C], BF16, name="vtt")
                        nc.sync.dma_start(vtt[:], vt_dram[t])
                        vtf = dbgp.tile([P, C], F32, name="vtf")
                        nc.vector.tensor_copy(out=vtf[:], in_=vtt[:])
                        nc.sync.dma_start(
                            out[128 * (t % 4) : 128 * (t % 4 + 1),
                                512 * (t // 4) : 512 * (t // 4) + 512],
                            vtf[:],
                        )
                elif dbg == "r":
                    d = dbgp.tile([P, NHW], F32, name="d")
                    nc.sync.dma_start(d[:], _bcast(r_dram[0, 0]))
                    nc.sync.dma_start(out[:P, :], d[:])
            return

        if os.environ.get("BASS_PHASE") == "A":
            with tc.tile_pool(name="dummy", bufs=1) as dummy:
                dmy = dummy.tile([P, CC, NHW], F32)
                nc.vector.tensor_copy(out=dmy[:], in_=xo_sb[:])
                nc.sync.dma_start(out[:].rearrange("(cc p) s -> p cc s", p=P), dmy[:])
            return

        # ================= phase B: attention + output =================
        with (
            tc.tile_pool(name="atp", bufs=3) as atp,
            tc.tile_pool(name="vload", bufs=4) as vload,
            tc.tile_pool(name="onorm", bufs=2) as onorm,
            tc.tile_pool(name="rcb", bufs=2) as rcb,
            tc.tile_pool(name="rows2", bufs=2) as rows2,
            tc.tile_pool(name="outst", bufs=2) as outst,
            tc.tile_pool(name="ppSc", bufs=2, space="PSUM") as ppSc,
            tc.tile_pool(name="ppO", bufs=1, space="PSUM") as ppO,
            tc.tile_pool(name="ppSum", bufs=2, space="PSUM") as ppSum,
        ):
            tails = []
            for sl in range(int(os.environ.get("BASS_SLICES", "2"))):  # sq slice of 512
                smax = 4 * sl + 3
                psum_o = [ppO.tile([P, 512], F32, tag=f"o{cc}", name=f"psum_o{cc}") for cc in range(CC)]
                psum_sum = ppSum.tile([1, 512], F32)
                for f in range(smax + 1):
                    qlo = max(P * f - 512 * sl, 0)
                    qw = 512 - qlo
                    for t in range(NT):
                        vt_t = vload.tile([P, C], BF16)
                        nc.sync.dma_start(vt_t[:], vt_dram[f * NT + t])

                        pss = ppSc.tile([P, 512], F32, tag="sc", name="pss")
                        for cc in range(CC):
                            nc.tensor.matmul(
                                pss[:, :qw],
                                lhsT=K_sb[:, cc, f * NHW + t * P : f * NHW + (t + 1) * P],
                                rhs=Q_sb[:, cc, sl * 512 + qlo : (sl + 1) * 512],
                                start=(cc == 0),
                                stop=(cc == CC - 1),
                            )
                        at = atp.tile([P, 512], BF16)
                        nc.scalar.activation(
                            at[:, :qw], pss[:, :qw], Act.Exp, scale=float(INV_SQRT_C)
                        )
                        if os.environ.get("BASS_DEBUG") == "at" and sl == 0 and f == 0:
                            atf = atp.tile([P, 512], F32, name="atf")
                            nc.vector.tensor_copy(out=atf[:], in_=at[:])
                            nc.sync.dma_start(
                                out[128 * (t % 4) : 128 * (t % 4 + 1),
                                    512 * (t // 4) : 512 * (t // 4) + 512],
                                atf[:],
                            )
                        first = f == 0 and t == 0
                        last = f == smax and t == NT - 1
                        nc.tensor.matmul(
                            psum_sum[:, qlo:],
                            lhsT=ones_bf[:],
                            rhs=at[:, :qw],
                            start=first,
                            stop=last,
                        )
                        for cc in range(CC):
                            nc.tensor.matmul(
                                psum_o[cc][:, qlo:],
                                lhsT=vt_t[:, cc * P : (cc + 1) * P],
                                rhs=at[:, :qw],
                                start=first,
                                stop=last,
                            )

                # 1/sum, broadcast via DRAM round-trip (concurrent with later work)
                rc = rows2.tile([1, 512], F32)
                nc.vector.reciprocal_approx_fast(out=rc[:], in_=psum_sum[:])
                nc.sync.dma_start(
                    rc_dram[:, sl * 512 : (sl + 1) * 512], rc[:]
                )
                rcB = rcb.tile([P, 512], F32)
                nc.sync.dma_start(rcB[:], _bcast(rc_dram[0, sl * 512 : (sl + 1) * 512]))

                if os.environ.get("BASS_DEBUG") == "sums" and sl == 0:
                    nc.sync.dma_start(out[:P, :512], rcB[:])
                if os.environ.get("BASS_DEBUG") == "o" and sl == 0:
                    for cc in range(CC):
                        of = rcb.tile([P, 512], F32, name="of")
                        nc.vector.tensor_copy(out=of[:], in_=psum_o[cc][:])
                        nc.sync.dma_start(out[cc * P : (cc + 1) * P, :512], of[:])
                # cast unnormalized o to bf16 (frees the psum_o banks early);
                # normalization is a per-column scale, commutes through WO
                onrm = onorm.tile([P, CC, 512], BF16)
                for cc in range(CC):
                    nc.scalar.activation(onrm[:, cc, :], psum_o[cc][:], Act.Copy)
                tails.append((sl, rcB, onrm))

            # deferred output projections: emitted after both slices' attention
            # so the PE never waits on a slice tail before starting the next
            for sl, rcB, onrm in tails:
                for oc in range(CC):
                    psw = ppSc.tile([P, 512], F32, tag="sc", name="psw")
                    for cc in range(CC):
                        nc.tensor.matmul(
                            psw[:],
                            lhsT=wo_sb[:, cc, oc * P : (oc + 1) * P],
                            rhs=onrm[:, cc, :],
                            start=(cc == 0),
                            stop=(cc == CC - 1),
                        )
                    o_sc = outst.tile([P, 512], F32, name="o_sc")
                    nc.vector.scalar_tensor_tensor(
                        out=o_sc[:],
                        in0=psw[:],
                        scalar=1.0,
                        in1=rcB[:],
                        op0=Alu.mult,
                        op1=Alu.mult,
                    )
                    o_out = outst.tile([P, 512], F32)
                    nc.vector.scalar_tensor_tensor(
                        out=o_out[:],
                        in0=o_sc[:],
                        scalar=boCol[:, oc : oc + 1],
                        in1=xo_sb[:, oc, sl * 512 : (sl + 1) * 512],
                        op0=Alu.add,
                        op1=Alu.add,
                    )
                    if os.environ.get("BASS_DEBUG", "") in ("", "final"):
                        nc.sync.dma_start(
                            out[oc * P : (oc + 1) * P, sl * 512 : (sl + 1) * 512], o_out[:]
                        )


def kernel(x, gamma, wq, bq, wk, bk, wv, bv, wo, bo):
    global LAST_RESULTS
    _ensure_axon_hooks()
    x = np.asarray(x, dtype=np.float32)
    gamma = np.asarray(gamma, dtype=np.float32).reshape(C)
    ws = {
        "wqT": np.ascontiguousarray(np.asarray(wq, np.float32).T).astype(ml_dtypes.bfloat16),
        "wkT": np.ascontiguousarray(np.asarray(wk, np.float32).T).astype(ml_dtypes.bfloat16),
        "wvT": np.ascontiguousarray(np.asarray(wv, np.float32).T).astype(ml_dtypes.bfloat16),
        "woT": np.ascontiguousarray(np.asarray(wo, np.float32).T).astype(ml_dtypes.bfloat16),
    }
    bs = {
        "bq": np.asarray(bq, np.float32).reshape(C),
        "bk": np.asarray(bk, np.float32).reshape(C),
        "bv": np.asarray(bv, np.float32).reshape(C),
        "bo": np.asarray(bo, np.float32).reshape(C),
    }

    xs = x.reshape(C, SEQ)  # [c, f*h*w], frame = s // 1024
    xf_bf = xs.astype(ml_dtypes.bfloat16)
    xsub = xs.reshape(C, NF, NHW // P, P)  # [c, frame, subchunk, 128]

    in_maps = []
    for i in range(NCORES):
        xo_i = np.ascontiguousarray(xsub[:, :, i, :]).reshape(C, NHW)
        in_maps.append(
            {"xf": xf_bf, "xo": xo_i, "gamma": gamma, **ws, **bs}
        )

    nc = _build_nc()
    nc.finalize()  # run Bacc passes (multi-wait splitting etc.) before lowering
    res = run_bass_kernel_spmd(nc, in_maps, list(range(NCORES)))
    LAST_RESULTS = res

    out_full = np.empty((C, SEQ), np.float32)
    ov = out_full.reshape(C, NF, NHW // P, P)
    for i in range(NCORES):
        ov[:, :, i, :] = res.results[i]["out"].reshape(C, NF, P)
    return out_full.reshape(B, C, NF, H, W)



# revision 3
# speedup vs baseline: 1.4236x; 1.4236x over previous
"""Trainium2 Bass kernel for AttnBlock (rmsnorm -> qkv -> block-causal frame
attention -> output proj -> residual).

Sharding (v2, sequence-parallel per the hint):
  * Queries: core i owns the i-th 128-token slice of every frame (8 query
    chunks of 128).  Query chunk s (frame s) attends frames 0..s, so spans are
    compile-time constants, identical on every core -> clean SPMD.
  * K/V projections: core i computes K/V ONLY for frame i (1024 tokens), then
    two AllGathers (K first, then V) distribute them in fp8e4 (transport +
    matmul dtype; scores here are tiny, sigma~0.2, so fp8 K/V error is ~1e-4).
  * While AG_K is in flight the PE does V/Q projections; while AG_V is in
    flight the PE runs slice-0 scores (lookahead, at-tiles buffered in SBUF)
    so the first AV matmul lands right as V arrives.

Matmul inputs bf16/fp8 (fp32 matmul is 4x slower); accumulation fp32 in PSUM.
Softmax skips max-subtraction (scores are O(0.2)); row sums via ones-vector
matmul on PE; 1/sum applied after the attn@V contraction (commutes with WO).
"""

import math
import os
import sys

import numpy as np

for _p in ("/opt/trn_rl_repo",):
    if _p not in sys.path:
        sys.path.insert(0, _p)

import ml_dtypes  # noqa: E402

import concourse.bass as bass  # noqa: E402
import concourse.tile as tile  # noqa: E402
from concourse import bacc  # noqa: E402
from concourse import mybir  # noqa: E402
from concourse.bass_utils import run_bass_kernel_spmd  # noqa: E402

BF16 = mybir.dt.bfloat16
FP8 = mybir.dt.float8e4
F32 = mybir.dt.float32

B, C, NF, H, W = 1, 512, 8, 32, 32
NHW = H * W          # 1024 tokens per frame
SEQ = NF * NHW       # 8192
NCORES = 8
P = 128              # partitions
CC = C // P          # 4 channel chunks
NT = NHW // P        # 8 key tiles per frame
SQRT_C = math.sqrt(C)
INV_SQRT_C = 1.0 / SQRT_C

KV_DT = FP8 if os.environ.get("BASS_KV_DTYPE", "fp8") == "fp8" else BF16

LAST_RESULTS = None  # BassKernelResults of the most recent run (for test.py)


def _ensure_axon_hooks():
    """bass_utils' trace path imports antenv.axon_hooks, which is absent from
    some container snapshots.  Provide the tiny registry (and wire the ctypes
    NTFF hook from trn_agent_boot when available) so tracing degrades
    gracefully instead of crashing."""
    import types

    try:
        import antenv.axon_hooks  # noqa: F401

        return
    except Exception:
        pass
    try:
        import antenv
    except Exception:
        antenv = types.ModuleType("antenv")
        sys.modules["antenv"] = antenv
    mod = types.ModuleType("antenv.axon_hooks")
    _h = [None]
    mod.set_axon_ntff_profile_hook = lambda hook: _h.__setitem__(0, hook)
    mod.get_axon_ntff_profile_hook = lambda: _h[0]
    sys.modules["antenv.axon_hooks"] = mod
    antenv.axon_hooks = mod
    try:
        from trn_agent_boot.trn_boot import _ntff_profile_via_ctypes

        hook = _ntff_profile_via_ctypes("/opt/axon/libaxon_pjrt.so")
        if hook is not None:
            mod.set_axon_ntff_profile_hook(hook)
    except Exception:
        pass


def _bcast(ap, p=P):
    """AP that reads a DRAM row and replicates it across p partitions."""
    return bass.AP(tensor=ap.tensor, offset=ap.offset, ap=[[0, p], *list(ap.ap)])


def _build_nc():
    nc = bacc.Bacc(num_devices=NCORES)

    xfi = nc.declare_dram_parameter("xfi", [C, NHW], BF16, isOutput=False)
    xo = nc.declare_dram_parameter("xo", [C, NHW], F32, isOutput=False)
    wqT = nc.declare_dram_parameter("wqT", [C, C], BF16, isOutput=False)
    wkT = nc.declare_dram_parameter("wkT", [C, C], BF16, isOutput=False)
    wvT = nc.declare_dram_parameter("wvT", [C, C], BF16, isOutput=False)
    woT = nc.declare_dram_parameter("woT", [C, C], BF16, isOutput=False)
    gamma = nc.declare_dram_parameter("gamma", [C], F32, isOutput=False)
    bq = nc.declare_dram_parameter("bq", [C], F32, isOutput=False)
    bk = nc.declare_dram_parameter("bk", [C], F32, isOutput=False)
    bv = nc.declare_dram_parameter("bv", [C], F32, isOutput=False)
    bo = nc.declare_dram_parameter("bo", [C], F32, isOutput=False)
    out = nc.declare_dram_parameter("out", [C, NHW], F32, isOutput=True)

    with tile.TileContext(nc) as tc:
        _emit(tc, xfi, xo, wqT, wkT, wvT, woT, gamma, bq, bk, bv, bo, out)
    return nc


def _emit(tc, xfi, xo, wqT, wkT, wvT, woT, gamma, bq, bk, bv, bo, out):
    nc = tc.nc
    Act = mybir.ActivationFunctionType
    Alu = mybir.AluOpType
    RG = [[i for i in range(NCORES)]]

    with (
        tc.tile_pool(name="dram", bufs=1, space="DRAM") as drp,
        tc.tile_pool(name="singles", bufs=1) as singles,
    ):
        # ---- DRAM scratch ----
        agk_in = drp.tile([C, NHW], KV_DT)
        agk_out = drp.tile([NCORES, C, NHW], KV_DT, addr_space="Shared")
        agv_in = drp.tile([NHW, C], KV_DT)
        agv_out = drp.tile([NCORES, NHW, C], KV_DT, addr_space="Shared")
        r_dram = drp.tile([2, 1, NHW], F32)    # rms scale rows (frame i, own)
        rc_dram = drp.tile([1, NHW], F32)      # softmax 1/sum row (own queries)

        # ---- constants & weights in SBUF ----
        def _wsb(wT):
            t = singles.tile([P, CC, C], BF16, tag=f"w_{wT.name}")
            nc.sync.dma_start(t[:], wT[:].rearrange("(cc p) o -> p cc o", p=P))
            return t

        def _col(v):
            t = singles.tile([P, CC], F32, tag=f"col_{v.name}")
            nc.sync.dma_start(t[:], v[:].rearrange("(cc p) -> p cc", p=P))
            return t

        gammaCol, bqCol, bkCol, boCol = _col(gamma), _col(bq), _col(bk), _col(bo)
        gsCol = singles.tile([P, CC], F32)          # gamma * sqrt(C)
        nc.vector.tensor_scalar_mul(gsCol[:], gammaCol[:], float(SQRT_C))

        bvB = singles.tile([P, C], F32)             # bv broadcast along partitions
        nc.sync.dma_start(bvB[:], _bcast(bv[:]))

        ones_bf = singles.tile([P, 1], BF16)
        nc.vector.memset(ones_bf[:], 1.0)
        eps_row = singles.tile([1, 1], F32)
        nc.vector.memset(eps_row[:], 1e-24)

        # ---- big persistent tensors ----
        K_sb = singles.tile([P, CC, SEQ], KV_DT)    # gathered keys [c_chunk, seq]
        Q_sb = singles.tile([P, CC, NHW], BF16)     # own queries
        xo_sb = singles.tile([P, CC, NHW], F32)     # own x (residual)
        at0_sb = singles.tile([P, 4 * NT, 512], BF16)  # slice-0 exp(scores) buffer

        # ================= phase A: norm + projections + gathers ============
        with (
            tc.tile_pool(name="stage", bufs=2) as stage,
            tc.tile_pool(name="hpool", bufs=2) as hpool,
            tc.tile_pool(name="rbpool", bufs=2) as rbpool,
            tc.tile_pool(name="rows", bufs=3) as rows,
            tc.tile_pool(name="vstage", bufs=4) as vstage,
            tc.tile_pool(name="ppA", bufs=4, space="PSUM") as ppA,
            tc.tile_pool(name="ppS", bufs=2, space="PSUM") as ppS,
        ):
            xfi_sb = stage.tile([P, CC, NHW], BF16, tag="xfi")
            nc.sync.dma_start(xfi_sb[:], xfi[:].rearrange("(cc p) s -> p cc s", p=P))
            wk_sb, wv_sb, wq_sb = _wsb(wkT), _wsb(wvT), _wsb(wqT)
            nc.sync.dma_start(xo_sb[:], xo[:].rearrange("(cc p) s -> p cc s", p=P))
            wo_sb = _wsb(woT)

            def _norm_half(x_in, ridx, pc):
                x2 = stage.tile([P, CC, 512], BF16, tag="x2")
                nc.scalar.activation(x2[:], x_in[:], Act.Square)
                ps = ppS.tile([1, 512], F32)
                for cc in range(CC):
                    nc.tensor.matmul(
                        ps[:],
                        lhsT=ones_bf[:],
                        rhs=x2[:, cc, :],
                        start=(cc == 0),
                        stop=(cc == CC - 1),
                    )
                nrm = rows.tile([1, 512], F32, tag="nrm")
                nc.scalar.activation(nrm[:], ps[:], Act.Sqrt, bias=eps_row[:])
                rrow = rows.tile([1, 512], F32, tag="rrow")
                nc.vector.reciprocal_approx_fast(out=rrow[:], in_=nrm[:])
                nc.sync.dma_start(r_dram[ridx, :, pc * 512 : (pc + 1) * 512], rrow[:])

            def _h_from(xsrc, ridx):
                rB = rbpool.tile([P, NHW], F32)
                nc.sync.dma_start(rB[:], _bcast(r_dram[ridx, 0]))
                ht = hpool.tile([P, CC, NHW], BF16)
                for cc in range(CC):
                    nc.vector.scalar_tensor_tensor(
                        out=ht[:, cc, :],
                        in0=xsrc[:, cc, :],
                        scalar=gsCol[:, cc : cc + 1],
                        in1=rB[:],
                        op0=Alu.mult,
                        op1=Alu.mult,
                    )
                return ht

            # ---- frame-i norm + h ----
            for pc in range(2):
                _norm_half(xfi_sb[:, :, pc * 512 : (pc + 1) * 512], 0, pc)
            h_i = _h_from(xfi_sb, 0)

            # ---- K_i projection -> agk_in -> AllGather K ----
            ksb_i = stage.tile([P, CC, NHW], KV_DT, tag="ksb")
            for oc in range(CC):
                for pc in range(2):
                    psk = ppA.tile([P, 512], F32, tag="ps", name="psk")
                    for cc in range(CC):
                        nc.tensor.matmul(
                            psk[:],
                            lhsT=wk_sb[:, cc, oc * P : (oc + 1) * P],
                            rhs=h_i[:, cc, pc * 512 : (pc + 1) * 512],
                            start=(cc == 0),
                            stop=(cc == CC - 1),
                        )
                    nc.scalar.activation(
                        ksb_i[:, oc, pc * 512 : (pc + 1) * 512],
                        psk[:],
                        Act.Identity,
                        bias=bkCol[:, oc : oc + 1],
                    )
            nc.sync.dma_start(
                agk_in[:].rearrange("(cc p) s -> p cc s", p=P), ksb_i[:]
            )
            nc.gpsimd.collective_compute(
                "AllGather",
                mybir.AluOpType.bypass,
                replica_groups=RG,
                ins=[agk_in[:].opt()],
                outs=[agk_out[:].opt()],
            )
            # gathered K -> SBUF, one DMA per frame (pipelines with attention)
            for j in range(NF):
                nc.sync.dma_start(
                    K_sb[:, :, j * NHW : (j + 1) * NHW],
                    agk_out[j].rearrange("(cc p) s -> p cc s", p=P),
                )

            # ---- V_i projection -> agv_in -> AllGather V ----
            for t in range(NT):
                psv = ppA.tile([P, 512], F32, tag="ps", name="psv")
                for cc in range(CC):
                    nc.tensor.matmul(
                        psv[:],
                        lhsT=h_i[:, cc, t * P : (t + 1) * P],
                        rhs=wv_sb[:, cc, :],
                        start=(cc == 0),
                        stop=(cc == CC - 1),
                    )
                vt_s = vstage.tile([P, C], KV_DT)
                nc.vector.scalar_tensor_tensor(
                    out=vt_s[:],
                    in0=psv[:],
                    scalar=1.0,
                    in1=bvB[:],
                    op0=Alu.mult,
                    op1=Alu.add,
                )
                nc.sync.dma_start(agv_in[t * P : (t + 1) * P, :], vt_s[:])
            nc.gpsimd.collective_compute(
                "AllGather",
                mybir.AluOpType.bypass,
                replica_groups=RG,
                ins=[agv_in[:].opt()],
                outs=[agv_out[:].opt()],
            )

            # ---- own norm + h + Q projection ----
            xbf = stage.tile([P, CC, NHW], BF16, tag="xbf")
            nc.vector.tensor_copy(out=xbf[:], in_=xo_sb[:])
            for pc in range(2):
                _norm_half(xbf[:, :, pc * 512 : (pc + 1) * 512], 1, pc)
            h_own = _h_from(xbf, 1)
            for oc in range(CC):
                for pc in range(2):
                    psq = ppA.tile([P, 512], F32, tag="ps", name="psq")
                    for cc in range(CC):
                        nc.tensor.matmul(
                            psq[:],
                            lhsT=wq_sb[:, cc, oc * P : (oc + 1) * P],
                            rhs=h_own[:, cc, pc * 512 : (pc + 1) * 512],
                            start=(cc == 0),
                            stop=(cc == CC - 1),
                        )
                    nc.scalar.activation(
                        Q_sb[:, oc, pc * 512 : (pc + 1) * 512],
                        psq[:],
                        Act.Identity,
                        bias=bqCol[:, oc : oc + 1],
                    )

        if os.environ.get("BASS_PHASE") == "A":
            with tc.tile_pool(name="dummy", bufs=1) as dummy:
                dmy = dummy.tile([P, CC, NHW], F32)
                nc.vector.tensor_copy(out=dmy[:], in_=xo_sb[:])
                nc.sync.dma_start(out[:].rearrange("(cc p) s -> p cc s", p=P), dmy[:])
            return

        # ================= phase B: attention + output =================
        with (
            tc.tile_pool(name="atp", bufs=6) as atp,
            tc.tile_pool(name="vload", bufs=6) as vload,
            tc.tile_pool(name="onorm", bufs=2) as onorm,
            tc.tile_pool(name="rcb", bufs=2) as rcb,
            tc.tile_pool(name="rows2", bufs=2) as rows2,
            tc.tile_pool(name="outst", bufs=2) as outst,
            tc.tile_pool(name="ppSc", bufs=2, space="PSUM") as ppSc,
            tc.tile_pool(name="ppO", bufs=1, space="PSUM") as ppO,
            tc.tile_pool(name="ppSum", bufs=2, space="PSUM") as ppSum,
        ):
            tails = []

            # ---------- slice 0 (query cols 0..511, frames 0..3) ----------
            # B1: scores+exp+rowsum only (buffered) so the PE keeps busy while
            # AG_V is still in flight; B2: the deferred attn@V matmuls.
            psum_o0 = [
                ppO.tile([P, 512], F32, tag=f"o{cc}", name=f"psum_o0{cc}")
                for cc in range(CC)
            ]
            psum_sum0 = ppSum.tile([1, 512], F32, tag="sum", name="psum_sum0")
            n0 = 0
            for f in range(4):
                qlo = P * f
                qw = 512 - qlo
                for t in range(NT):
                    pss = ppSc.tile([P, 512], F32, tag="sc", name="pss")
                    for cc in range(CC):
                        nc.tensor.matmul(
                            pss[:, :qw],
                            lhsT=K_sb[:, cc, f * NHW + t * P : f * NHW + (t + 1) * P],
                            rhs=Q_sb[:, cc, qlo:512],
                            start=(cc == 0),
                            stop=(cc == CC - 1),
                        )
                    nc.scalar.activation(
                        at0_sb[:, n0, :qw], pss[:, :qw], Act.Exp,
                        scale=float(INV_SQRT_C),
                    )
                    nc.tensor.matmul(
                        psum_sum0[:, qlo:],
                        lhsT=ones_bf[:],
                        rhs=at0_sb[:, n0, :qw],
                        start=(n0 == 0),
                        stop=(n0 == 4 * NT - 1),
                    )
                    n0 += 1
            rc0 = rows2.tile([1, 512], F32)
            nc.vector.reciprocal_approx_fast(out=rc0[:], in_=psum_sum0[:])
            nc.sync.dma_start(rc_dram[:, 0:512], rc0[:])
            rcB0 = rcb.tile([P, 512], F32)
            nc.sync.dma_start(rcB0[:], _bcast(rc_dram[0, 0:512]))
            n0 = 0
            for f in range(4):
                qlo = P * f
                qw = 512 - qlo
                for t in range(NT):
                    vt_t = vload.tile([P, C], KV_DT)
                    nc.sync.dma_start(vt_t[:], agv_out[f, t * P : (t + 1) * P, :])
                    for cc in range(CC):
                        nc.tensor.matmul(
                            psum_o0[cc][:, qlo:],
                            lhsT=vt_t[:, cc * P : (cc + 1) * P],
                            rhs=at0_sb[:, n0, :qw],
                            start=(n0 == 0),
                            stop=(n0 == 4 * NT - 1),
                        )
                    n0 += 1
            onrm0 = onorm.tile([P, CC, 512], BF16)
            for cc in range(CC):
                nc.scalar.activation(onrm0[:, cc, :], psum_o0[cc][:], Act.Copy)
            tails.append((0, rcB0, onrm0))

            # ---------- slice 1 (query cols 512..1023, frames 0..7) ----------
            psum_o1 = [
                ppO.tile([P, 512], F32, tag=f"o{cc}", name=f"psum_o1{cc}")
                for cc in range(CC)
            ]
            psum_sum1 = ppSum.tile([1, 512], F32, tag="sum", name="psum_sum1")
            for f in range(NF):
                qlo = max(P * f - 512, 0)
                qw = 512 - qlo
                for t in range(NT):
                    vt_t = vload.tile([P, C], KV_DT)
                    nc.sync.dma_start(vt_t[:], agv_out[f, t * P : (t + 1) * P, :])

                    pss = ppSc.tile([P, 512], F32, tag="sc", name="pss")
                    for cc in range(CC):
                        nc.tensor.matmul(
                            pss[:, :qw],
                            lhsT=K_sb[:, cc, f * NHW + t * P : f * NHW + (t + 1) * P],
                            rhs=Q_sb[:, cc, 512 + qlo : 1024],
                            start=(cc == 0),
                            stop=(cc == CC - 1),
                        )
                    at = atp.tile([P, 512], BF16)
                    nc.scalar.activation(
                        at[:, :qw], pss[:, :qw], Act.Exp, scale=float(INV_SQRT_C)
                    )
                    first = f == 0 and t == 0
                    last = f == NF - 1 and t == NT - 1
                    nc.tensor.matmul(
                        psum_sum1[:, qlo:],
                        lhsT=ones_bf[:],
                        rhs=at[:, :qw],
                        start=first,
                        stop=last,
                    )
                    for cc in range(CC):
                        nc.tensor.matmul(
                            psum_o1[cc][:, qlo:],
                            lhsT=vt_t[:, cc * P : (cc + 1) * P],
                            rhs=at[:, :qw],
                            start=first,
                            stop=last,
                        )
            rc1 = rows2.tile([1, 512], F32)
            nc.vector.reciprocal_approx_fast(out=rc1[:], in_=psum_sum1[:])
            nc.sync.dma_start(rc_dram[:, 512:1024], rc1[:])
            rcB1 = rcb.tile([P, 512], F32)
            nc.sync.dma_start(rcB1[:], _bcast(rc_dram[0, 512:1024]))
            onrm1 = onorm.tile([P, CC, 512], BF16)
            for cc in range(CC):
                nc.scalar.activation(onrm1[:, cc, :], psum_o1[cc][:], Act.Copy)
            tails.append((1, rcB1, onrm1))

            # deferred output projections: emitted after both slices' attention
            # so the PE never waits on a slice tail before starting the next
            for sl, rcB, onrm in tails:
                for oc in range(CC):
                    psw = ppSc.tile([P, 512], F32, tag="sc", name="psw")
                    for cc in range(CC):
                        nc.tensor.matmul(
                            psw[:],
                            lhsT=wo_sb[:, cc, oc * P : (oc + 1) * P],
                            rhs=onrm[:, cc, :],
                            start=(cc == 0),
                            stop=(cc == CC - 1),
                        )
                    o_sc = outst.tile([P, 512], F32, name="o_sc")
                    nc.vector.scalar_tensor_tensor(
                        out=o_sc[:],
                        in0=psw[:],
                        scalar=1.0,
                        in1=rcB[:],
                        op0=Alu.mult,
                        op1=Alu.mult,
                    )
                    o_out = outst.tile([P, 512], F32)
                    nc.vector.scalar_tensor_tensor(
                        out=o_out[:],
                        in0=o_sc[:],
                        scalar=boCol[:, oc : oc + 1],
                        in1=xo_sb[:, oc, sl * 512 : (sl + 1) * 512],
                        op0=Alu.add,
                        op1=Alu.add,
                    )
                    nc.sync.dma_start(
                        out[oc * P : (oc + 1) * P, sl * 512 : (sl + 1) * 512], o_out[:]
                    )


def kernel(x, gamma, wq, bq, wk, bk, wv, bv, wo, bo):
    global LAST_RESULTS
    _ensure_axon_hooks()
    x = np.asarray(x, dtype=np.float32)
    gamma = np.asarray(gamma, dtype=np.float32).reshape(C)
    ws = {
        "wqT": np.ascontiguousarray(np.asarray(wq, np.float32).T).astype(ml_dtypes.bfloat16),
        "wkT": np.ascontiguousarray(np.asarray(wk, np.float32).T).astype(ml_dtypes.bfloat16),
        "wvT": np.ascontiguousarray(np.asarray(wv, np.float32).T).astype(ml_dtypes.bfloat16),
        "woT": np.ascontiguousarray(np.asarray(wo, np.float32).T).astype(ml_dtypes.bfloat16),
    }
    bs = {
        "bq": np.asarray(bq, np.float32).reshape(C),
        "bk": np.asarray(bk, np.float32).reshape(C),
        "bv": np.asarray(bv, np.float32).reshape(C),
        "bo": np.asarray(bo, np.float32).reshape(C),
    }

    xs = x.reshape(C, SEQ)  # [c, f*h*w], frame = s // 1024
    xsub = xs.reshape(C, NF, NHW // P, P)  # [c, frame, subchunk, 128]

    in_maps = []
    for i in range(NCORES):
        xo_i = np.ascontiguousarray(xsub[:, :, i, :]).reshape(C, NHW)
        xfi_i = np.ascontiguousarray(xs[:, i * NHW : (i + 1) * NHW]).astype(
            ml_dtypes.bfloat16
        )
        in_maps.append(
            {"xfi": xfi_i, "xo": xo_i, "gamma": gamma, **ws, **bs}
        )

    nc = _build_nc()
    nc.finalize()  # run Bacc passes (multi-wait splitting etc.) before lowering
    res = run_bass_kernel_spmd(nc, in_maps, list(range(NCORES)))
    LAST_RESULTS = res

    out_full = np.empty((C, SEQ), np.float32)
    ov = out_full.reshape(C, NF, NHW // P, P)
    for i in range(NCORES):
        ov[:, :, i, :] = res.results[i]["out"].reshape(C, NF, P)
    return out_full.reshape(B, C, NF, H, W)


# revision 8
# speedup vs baseline: 1.4893x; 1.0462x over previous
"""Trainium2 Bass kernel for AttnBlock (rmsnorm -> qkv -> block-causal frame
attention -> output proj -> residual).

Sharding (v2, sequence-parallel per the hint):
  * Queries: core i owns the i-th 128-token slice of every frame (8 query
    chunks of 128).  Query chunk s (frame s) attends frames 0..s, so spans are
    compile-time constants, identical on every core -> clean SPMD.
  * K/V projections: core i computes K/V ONLY for frame i (1024 tokens), then
    two AllGathers (K first, then V) distribute them in fp8e4 (transport +
    matmul dtype; scores here are tiny, sigma~0.2, so fp8 K/V error is ~1e-4).
  * While AG_K is in flight the PE does V/Q projections; while AG_V is in
    flight the PE runs slice-0 scores (lookahead, at-tiles buffered in SBUF)
    so the first AV matmul lands right as V arrives.

Matmul inputs bf16/fp8 (fp32 matmul is 4x slower); accumulation fp32 in PSUM.
Softmax skips max-subtraction (scores are O(0.2)); row sums via ones-vector
matmul on PE; 1/sum applied after the attn@V contraction (commutes with WO).
"""

import math
import os
import sys

import numpy as np

for _p in ("/opt/trn_rl_repo",):
    if _p not in sys.path:
        sys.path.insert(0, _p)

import ml_dtypes  # noqa: E402

import concourse.bass as bass  # noqa: E402
import concourse.tile as tile  # noqa: E402
from concourse import bacc  # noqa: E402
from concourse import mybir  # noqa: E402
from concourse.bass_utils import run_bass_kernel_spmd  # noqa: E402

BF16 = mybir.dt.bfloat16
FP8 = mybir.dt.float8e4
F32 = mybir.dt.float32

B, C, NF, H, W = 1, 512, 8, 32, 32
NHW = H * W          # 1024 tokens per frame
SEQ = NF * NHW       # 8192
NCORES = 8
P = 128              # partitions
CC = C // P          # 4 channel chunks
NT = NHW // P        # 8 key tiles per frame
SQRT_C = math.sqrt(C)
INV_SQRT_C = 1.0 / SQRT_C

KV_DT = FP8 if os.environ.get("BASS_KV_DTYPE", "fp8") == "fp8" else BF16

LAST_RESULTS = None  # BassKernelResults of the most recent run (for test.py)


def _ensure_axon_hooks():
    """bass_utils' trace path imports antenv.axon_hooks, which is absent from
    some container snapshots.  Provide the tiny registry (and wire the ctypes
    NTFF hook from trn_agent_boot when available) so tracing degrades
    gracefully instead of crashing."""
    import types

    try:
        import antenv.axon_hooks  # noqa: F401

        return
    except Exception:
        pass
    try:
        import antenv
    except Exception:
        antenv = types.ModuleType("antenv")
        sys.modules["antenv"] = antenv
    mod = types.ModuleType("antenv.axon_hooks")
    _h = [None]
    mod.set_axon_ntff_profile_hook = lambda hook: _h.__setitem__(0, hook)
    mod.get_axon_ntff_profile_hook = lambda: _h[0]
    sys.modules["antenv.axon_hooks"] = mod
    antenv.axon_hooks = mod
    try:
        from trn_agent_boot.trn_boot import _ntff_profile_via_ctypes

        hook = _ntff_profile_via_ctypes("/opt/axon/libaxon_pjrt.so")
        if hook is not None:
            mod.set_axon_ntff_profile_hook(hook)
    except Exception:
        pass


def _bcast(ap, p=P):
    """AP that reads a DRAM row and replicates it across p partitions."""
    return bass.AP(tensor=ap.tensor, offset=ap.offset, ap=[[0, p], *list(ap.ap)])


def _build_nc():
    nc = bacc.Bacc(num_devices=NCORES)

    xfi = nc.declare_dram_parameter("xfi", [C, NHW], BF16, isOutput=False)
    xo = nc.declare_dram_parameter("xo", [C, NHW], F32, isOutput=False)
    wqT = nc.declare_dram_parameter("wqT", [C, C], BF16, isOutput=False)
    wkT = nc.declare_dram_parameter("wkT", [C, C], BF16, isOutput=False)
    wvT = nc.declare_dram_parameter("wvT", [C, C], BF16, isOutput=False)
    woT = nc.declare_dram_parameter("woT", [C, C], BF16, isOutput=False)
    gamma = nc.declare_dram_parameter("gamma", [C], F32, isOutput=False)
    bq = nc.declare_dram_parameter("bq", [C], F32, isOutput=False)
    bk = nc.declare_dram_parameter("bk", [C], F32, isOutput=False)
    bv = nc.declare_dram_parameter("bv", [C], F32, isOutput=False)
    bo = nc.declare_dram_parameter("bo", [C], F32, isOutput=False)
    out = nc.declare_dram_parameter("out", [C, NHW], F32, isOutput=True)

    with tile.TileContext(nc) as tc:
        _emit(tc, xfi, xo, wqT, wkT, wvT, woT, gamma, bq, bk, bv, bo, out)
    return nc


def _emit(tc, xfi, xo, wqT, wkT, wvT, woT, gamma, bq, bk, bv, bo, out):
    nc = tc.nc
    Act = mybir.ActivationFunctionType
    Alu = mybir.AluOpType
    RG = [[i for i in range(NCORES)]]

    with (
        tc.tile_pool(name="dram", bufs=1, space="DRAM") as drp,
        tc.tile_pool(name="singles", bufs=1) as singles,
    ):
        # ---- DRAM scratch ----
        agk_in = drp.tile([C, NHW], KV_DT)
        agk_out = drp.tile([NCORES, C, NHW], KV_DT, addr_space="Shared")
        agv_in = drp.tile([NHW, C], KV_DT)
        agv_out = drp.tile([NCORES, NHW, C], KV_DT, addr_space="Shared")
        r_dram = drp.tile([2, 1, NHW], F32)    # rms scale rows (frame i, own)
        rc_dram = drp.tile([1, NHW], F32)      # softmax 1/sum row (own queries)

        # ---- constants & weights in SBUF ----
        def _wsb(wT):
            t = singles.tile([P, CC, C], BF16, tag=f"w_{wT.name}")
            nc.sync.dma_start(t[:], wT[:].rearrange("(cc p) o -> p cc o", p=P))
            return t

        def _col(v):
            t = singles.tile([P, CC], F32, tag=f"col_{v.name}")
            nc.sync.dma_start(t[:], v[:].rearrange("(cc p) -> p cc", p=P))
            return t

        gammaCol, bqCol, bkCol, boCol = _col(gamma), _col(bq), _col(bk), _col(bo)
        gsCol = singles.tile([P, CC], F32)          # gamma * sqrt(C)
        nc.vector.tensor_scalar_mul(gsCol[:], gammaCol[:], float(SQRT_C))

        bvB = singles.tile([P, C], F32)             # bv broadcast along partitions
        nc.sync.dma_start(bvB[:], _bcast(bv[:]))

        ones_bf = singles.tile([P, 1], BF16)
        nc.vector.memset(ones_bf[:], 1.0)
        ones_row = singles.tile([1, P], F32)
        nc.vector.memset(ones_row[:], 1.0)
        eps_row = singles.tile([1, 1], F32)
        nc.vector.memset(eps_row[:], 1e-24)

        # ---- big persistent tensors ----
        K_sb = singles.tile([P, CC, SEQ], KV_DT)    # gathered keys [c_chunk, seq]
        Q_sb = singles.tile([P, CC, NHW], BF16)     # own queries
        xo_sb = singles.tile([P, CC, NHW], F32)     # own x (residual)
        at0_sb = singles.tile([P, 4 * NT, 512], BF16)  # slice-0 exp(scores) buffer

        # ================= phase A: norm + projections + gathers ============
        with (
            tc.tile_pool(name="stage", bufs=2) as stage,
            tc.tile_pool(name="hpool", bufs=2) as hpool,
            tc.tile_pool(name="rows", bufs=3) as rows,
            tc.tile_pool(name="vstage", bufs=4) as vstage,
            tc.tile_pool(name="ppA", bufs=4, space="PSUM") as ppA,
            tc.tile_pool(name="ppS", bufs=2, space="PSUM") as ppS,
            tc.tile_pool(name="ppR", bufs=2, space="PSUM") as ppR,
        ):
            # DMA priority order: xfi is the critical path to AG_K, then the
            # weights in first-use order; xo/wq/wo only matter mid-gather.
            xfi_sb = stage.tile([P, CC, NHW], BF16, tag="xfi")
            nc.sync.dma_start(xfi_sb[:], xfi[:].rearrange("(cc p) s -> p cc s", p=P))
            wk_sb, wv_sb = _wsb(wkT), _wsb(wvT)

            def _norm_half(x_in, pc):
                """rms rows for a 512-token half -> [P, 512] broadcast in PSUM
                (ones-matmul broadcast; no DRAM round-trip)."""
                x2 = stage.tile([P, CC, 512], BF16, tag="x2")
                nc.scalar.activation(x2[:], x_in[:], Act.Square)
                ps = ppS.tile([1, 512], F32)
                for cc in range(CC):
                    nc.tensor.matmul(
                        ps[:],
                        lhsT=ones_bf[:],
                        rhs=x2[:, cc, :],
                        start=(cc == 0),
                        stop=(cc == CC - 1),
                    )
                nrm = rows.tile([1, 512], F32, tag="nrm")
                nc.scalar.activation(nrm[:], ps[:], Act.Sqrt, bias=eps_row[:])
                rrow = rows.tile([1, 512], F32, tag="rrow")
                nc.vector.reciprocal_approx_fast(out=rrow[:], in_=nrm[:])
                rB = ppR.tile([P, 512], F32)
                nc.tensor.matmul(rB[:], lhsT=ones_row[:], rhs=rrow[:])
                return rB

            def _h_from(xsrc):
                ht = hpool.tile([P, CC, NHW], BF16)
                for pc in range(2):
                    rB = _norm_half(xsrc[:, :, pc * 512 : (pc + 1) * 512], pc)
                    for cc in range(CC):
                        nc.vector.scalar_tensor_tensor(
                            out=ht[:, cc, pc * 512 : (pc + 1) * 512],
                            in0=xsrc[:, cc, pc * 512 : (pc + 1) * 512],
                            scalar=gsCol[:, cc : cc + 1],
                            in1=rB[:],
                            op0=Alu.mult,
                            op1=Alu.mult,
                        )
                return ht

            h_i = _h_from(xfi_sb)

            # ---- K_i projection -> agk_in (half-granularity) -> AllGather K
            ksb_i = stage.tile([P, CC, NHW], KV_DT, tag="ksb")
            agk_in_r = agk_in[:].rearrange("(cc p) s -> p cc s", p=P)
            for pc in range(2):
                for oc in range(CC):
                    psk = ppA.tile([P, 512], F32, tag="ps", name="psk")
                    for cc in range(CC):
                        nc.tensor.matmul(
                            psk[:],
                            lhsT=wk_sb[:, cc, oc * P : (oc + 1) * P],
                            rhs=h_i[:, cc, pc * 512 : (pc + 1) * 512],
                            start=(cc == 0),
                            stop=(cc == CC - 1),
                        )
                    nc.scalar.activation(
                        ksb_i[:, oc, pc * 512 : (pc + 1) * 512],
                        psk[:],
                        Act.Identity,
                        bias=bkCol[:, oc : oc + 1],
                    )
                nc.sync.dma_start(
                    agk_in_r[:, :, pc * 512 : (pc + 1) * 512],
                    ksb_i[:, :, pc * 512 : (pc + 1) * 512],
                )
            nc.gpsimd.collective_compute(
                "AllGather",
                mybir.AluOpType.bypass,
                replica_groups=RG,
                ins=[agk_in[:].opt()],
                outs=[agk_out[:].opt()],
            )

            # ---- V_i projection -> agv_in -> AllGather V ----
            for t in range(NT):
                psv = ppA.tile([P, 512], F32, tag="ps", name="psv")
                for cc in range(CC):
                    nc.tensor.matmul(
                        psv[:],
                        lhsT=h_i[:, cc, t * P : (t + 1) * P],
                        rhs=wv_sb[:, cc, :],
                        start=(cc == 0),
                        stop=(cc == CC - 1),
                    )
                vt_s = vstage.tile([P, C], KV_DT)
                nc.vector.scalar_tensor_tensor(
                    out=vt_s[:],
                    in0=psv[:],
                    scalar=1.0,
                    in1=bvB[:],
                    op0=Alu.mult,
                    op1=Alu.add,
                )
                nc.sync.dma_start(agv_in[t * P : (t + 1) * P, :], vt_s[:])
            nc.gpsimd.collective_compute(
                "AllGather",
                mybir.AluOpType.bypass,
                replica_groups=RG,
                ins=[agv_in[:].opt()],
                outs=[agv_out[:].opt()],
            )

            # ---- own norm + h + Q projection (fills the gather window) ----
            nc.sync.dma_start(xo_sb[:], xo[:].rearrange("(cc p) s -> p cc s", p=P))
            wq_sb, wo_sb = _wsb(wqT), _wsb(woT)
            xbf = stage.tile([P, CC, NHW], BF16, tag="xbf")
            nc.vector.tensor_copy(out=xbf[:], in_=xo_sb[:])
            h_own = _h_from(xbf)
            for oc in range(CC):
                for pc in range(2):
                    psq = ppA.tile([P, 512], F32, tag="ps", name="psq")
                    for cc in range(CC):
                        nc.tensor.matmul(
                            psq[:],
                            lhsT=wq_sb[:, cc, oc * P : (oc + 1) * P],
                            rhs=h_own[:, cc, pc * 512 : (pc + 1) * 512],
                            start=(cc == 0),
                            stop=(cc == CC - 1),
                        )
                    nc.scalar.activation(
                        Q_sb[:, oc, pc * 512 : (pc + 1) * 512],
                        psq[:],
                        Act.Identity,
                        bias=bqCol[:, oc : oc + 1],
                    )

            # gathered K -> SBUF, one DMA per frame (pipelines with attention;
            # emitted last so no phase-A DMA queues behind the AG_K wait)
            for j in range(NF):
                nc.sync.dma_start(
                    K_sb[:, :, j * NHW : (j + 1) * NHW],
                    agk_out[j].rearrange("(cc p) s -> p cc s", p=P),
                )

        if os.environ.get("BASS_PHASE") == "A":
            with tc.tile_pool(name="dummy", bufs=1) as dummy:
                dmy = dummy.tile([P, CC, NHW], F32)
                nc.vector.tensor_copy(out=dmy[:], in_=xo_sb[:])
                nc.sync.dma_start(out[:].rearrange("(cc p) s -> p cc s", p=P), dmy[:])
            return

        # ================= phase B: attention + output =================
        with (
            tc.tile_pool(name="atp", bufs=6) as atp,
            tc.tile_pool(name="vload", bufs=6) as vload,
            tc.tile_pool(name="onorm", bufs=2) as onorm,
            tc.tile_pool(name="rcb", bufs=2) as rcb,
            tc.tile_pool(name="rows2", bufs=2) as rows2,
            tc.tile_pool(name="outst", bufs=2) as outst,
            tc.tile_pool(name="ppSc", bufs=2, space="PSUM") as ppSc,
            tc.tile_pool(name="ppO", bufs=1, space="PSUM") as ppO,
            tc.tile_pool(name="ppSum", bufs=2, space="PSUM") as ppSum,
        ):
            def _tail(sl, rcB, onrm):
                for oc in range(CC):
                    psw = ppSc.tile([P, 512], F32, tag="sc", name="psw")
                    for cc in range(CC):
                        nc.tensor.matmul(
                            psw[:],
                            lhsT=wo_sb[:, cc, oc * P : (oc + 1) * P],
                            rhs=onrm[:, cc, :],
                            start=(cc == 0),
                            stop=(cc == CC - 1),
                        )
                    o_sc = outst.tile([P, 512], F32, name="o_sc")
                    nc.vector.scalar_tensor_tensor(
                        out=o_sc[:],
                        in0=psw[:],
                        scalar=1.0,
                        in1=rcB[:],
                        op0=Alu.mult,
                        op1=Alu.mult,
                    )
                    o_out = outst.tile([P, 512], F32)
                    nc.vector.scalar_tensor_tensor(
                        out=o_out[:],
                        in0=o_sc[:],
                        scalar=boCol[:, oc : oc + 1],
                        in1=xo_sb[:, oc, sl * 512 : (sl + 1) * 512],
                        op0=Alu.add,
                        op1=Alu.add,
                    )
                    nc.sync.dma_start(
                        out[oc * P : (oc + 1) * P, sl * 512 : (sl + 1) * 512], o_out[:]
                    )

            # ---------- slice 0 (query cols 0..511, frames 0..3) ----------
            # B1: scores+exp+rowsum only (buffered) so the PE keeps busy while
            # AG_V is still in flight; B2: the deferred attn@V matmuls.
            psum_o0 = [
                ppO.tile([P, 512], F32, tag=f"o{cc}", name=f"psum_o0{cc}")
                for cc in range(CC)
            ]
            psum_sum0 = ppSum.tile([1, 512], F32, tag="sum", name="psum_sum0")
            n0 = 0
            for f in range(4):
                qlo = P * f
                qw = 512 - qlo
                for t in range(NT):
                    pss = ppSc.tile([P, 512], F32, tag="sc", name="pss")
                    for cc in range(CC):
                        nc.tensor.matmul(
                            pss[:, :qw],
                            lhsT=K_sb[:, cc, f * NHW + t * P : f * NHW + (t + 1) * P],
                            rhs=Q_sb[:, cc, qlo:512],
                            start=(cc == 0),
                            stop=(cc == CC - 1),
                        )
                    nc.scalar.activation(
                        at0_sb[:, n0, :qw], pss[:, :qw], Act.Exp,
                        scale=float(INV_SQRT_C),
                    )
                    nc.tensor.matmul(
                        psum_sum0[:, qlo:],
                        lhsT=ones_bf[:],
                        rhs=at0_sb[:, n0, :qw],
                        start=(n0 == 0),
                        stop=(n0 == 4 * NT - 1),
                    )
                    n0 += 1
            rc0 = rows2.tile([1, 512], F32)
            nc.vector.reciprocal_approx_fast(out=rc0[:], in_=psum_sum0[:])
            nc.sync.dma_start(rc_dram[:, 0:512], rc0[:])
            rcB0 = rcb.tile([P, 512], F32)
            nc.sync.dma_start(rcB0[:], _bcast(rc_dram[0, 0:512]))
            n0 = 0
            for f in range(4):
                qlo = P * f
                qw = 512 - qlo
                for t in range(NT):
                    vt_t = vload.tile([P, C], KV_DT)
                    nc.sync.dma_start(vt_t[:], agv_out[f, t * P : (t + 1) * P, :])
                    for cc in range(CC):
                        nc.tensor.matmul(
                            psum_o0[cc][:, qlo:],
                            lhsT=vt_t[:, cc * P : (cc + 1) * P],
                            rhs=at0_sb[:, n0, :qw],
                            start=(n0 == 0),
                            stop=(n0 == 4 * NT - 1),
                        )
                    n0 += 1
            onrm0 = onorm.tile([P, CC, 512], BF16)
            for cc in range(CC):
                nc.scalar.activation(onrm0[:, cc, :], psum_o0[cc][:], Act.Copy)
            _tail(0, rcB0, onrm0)

            # ---------- slice 1 (query cols 512..1023, frames 0..7) ----------
            psum_o1 = [
                ppO.tile([P, 512], F32, tag=f"o{cc}", name=f"psum_o1{cc}")
                for cc in range(CC)
            ]
            psum_sum1 = ppSum.tile([1, 512], F32, tag="sum", name="psum_sum1")
            for f in range(NF):
                qlo = max(P * f - 512, 0)
                qw = 512 - qlo
                for t in range(NT):
                    vt_t = vload.tile([P, C], KV_DT)
                    nc.sync.dma_start(vt_t[:], agv_out[f, t * P : (t + 1) * P, :])

                    pss = ppSc.tile([P, 512], F32, tag="sc", name="pss")
                    for cc in range(CC):
                        nc.tensor.matmul(
                            pss[:, :qw],
                            lhsT=K_sb[:, cc, f * NHW + t * P : f * NHW + (t + 1) * P],
                            rhs=Q_sb[:, cc, 512 + qlo : 1024],
                            start=(cc == 0),
                            stop=(cc == CC - 1),
                        )
                    at = atp.tile([P, 512], BF16)
                    nc.scalar.activation(
                        at[:, :qw], pss[:, :qw], Act.Exp, scale=float(INV_SQRT_C)
                    )
                    first = f == 0 and t == 0
                    last = f == NF - 1 and t == NT - 1
                    nc.tensor.matmul(
                        psum_sum1[:, qlo:],
                        lhsT=ones_bf[:],
                        rhs=at[:, :qw],
                        start=first,
                        stop=last,
                    )
                    for cc in range(CC):
                        nc.tensor.matmul(
                            psum_o1[cc][:, qlo:],
                            lhsT=vt_t[:, cc * P : (cc + 1) * P],
                            rhs=at[:, :qw],
                            start=first,
                            stop=last,
                        )
            rc1 = rows2.tile([1, 512], F32)
            nc.vector.reciprocal_approx_fast(out=rc1[:], in_=psum_sum1[:])
            nc.sync.dma_start(rc_dram[:, 512:1024], rc1[:])
            rcB1 = rcb.tile([P, 512], F32)
            nc.sync.dma_start(rcB1[:], _bcast(rc_dram[0, 512:1024]))
            onrm1 = onorm.tile([P, CC, 512], BF16)
            for cc in range(CC):
                nc.scalar.activation(onrm1[:, cc, :], psum_o1[cc][:], Act.Copy)
            _tail(1, rcB1, onrm1)


def kernel(x, gamma, wq, bq, wk, bk, wv, bv, wo, bo):
    global LAST_RESULTS
    _ensure_axon_hooks()
    x = np.asarray(x, dtype=np.float32)
    gamma = np.asarray(gamma, dtype=np.float32).reshape(C)
    ws = {
        "wqT": np.ascontiguousarray(np.asarray(wq, np.float32).T).astype(ml_dtypes.bfloat16),
        "wkT": np.ascontiguousarray(np.asarray(wk, np.float32).T).astype(ml_dtypes.bfloat16),
        "wvT": np.ascontiguousarray(np.asarray(wv, np.float32).T).astype(ml_dtypes.bfloat16),
        "woT": np.ascontiguousarray(np.asarray(wo, np.float32).T).astype(ml_dtypes.bfloat16),
    }
    bs = {
        "bq": np.asarray(bq, np.float32).reshape(C),
        "bk": np.asarray(bk, np.float32).reshape(C),
        "bv": np.asarray(bv, np.float32).reshape(C),
        "bo": np.asarray(bo, np.float32).reshape(C),
    }

    xs = x.reshape(C, SEQ)  # [c, f*h*w], frame = s // 1024
    xsub = xs.reshape(C, NF, NHW // P, P)  # [c, frame, subchunk, 128]

    in_maps = []
    for i in range(NCORES):
        xo_i = np.ascontiguousarray(xsub[:, :, i, :]).reshape(C, NHW)
        xfi_i = np.ascontiguousarray(xs[:, i * NHW : (i + 1) * NHW]).astype(
            ml_dtypes.bfloat16
        )
        in_maps.append(
            {"xfi": xfi_i, "xo": xo_i, "gamma": gamma, **ws, **bs}
        )

    nc = _build_nc()
    nc.finalize()  # run Bacc passes (multi-wait splitting etc.) before lowering
    res = run_bass_kernel_spmd(nc, in_maps, list(range(NCORES)))
    LAST_RESULTS = res

    out_full = np.empty((C, SEQ), np.float32)
    ov = out_full.reshape(C, NF, NHW // P, P)
    for i in range(NCORES):
        ov[:, :, i, :] = res.results[i]["out"].reshape(C, NF, P)
    return out_full.reshape(B, C, NF, H, W)
